# revision 2
# baseline (speedup 1.0000x reference)
"""Trainium2 Bass kernel for nn_CADenseMul.

Math (see reference):
    chi  = sigmoid(context @ W + Bc)          # [B, R]
    s    = S * chi                            # [B, R]
    out  = ((inputs @ U) * s) @ V.T + bias    # [B, UNITS]

Strategy v2:
  - Data-parallel over batch B across 8 cores (B=4096 -> 512 rows/core).
  - Host-side prep: per-core transposed activation shards + params packed
    into ONE [128, cols] bf16 blob -> ONE input DMA.  All compute is gated
    on that single DMA completion, so the entire input load overlaps the
    framework startup phase (before the first datapath instruction).
  - Device (transposed-activation layout, batch as free dim):
        h.T    = W.T @ ctx.T          (PSUM; sigmoid+Bc on ACT)
        proj.T = U_s.T @ x.T          (b-halves of 256)
        psT    = proj.T * chi.T       (DVE, cast bf16)
        out    = psT.T @ V.T          (psT stationary, natural-layout out)
  - Output stored bf16; host concats, adds bias fp32.
"""

import numpy as np
import ml_dtypes

import concourse.bass as bass
import concourse.tile as tile
from concourse import bacc, mybir
from concourse.bass_utils import run_bass_kernel_spmd

N_CORES = 8
B, D_IN, D_CTX, UNITS, R = 4096, 2048, 512, 2048, 256
BS = B // N_CORES        # 512 batch rows per core
KT_X = D_IN // 128       # 16
KT_C = D_CTX // 128      # 4
RT = R // 128            # 2
NBT = BS // 128          # 4 output batch tiles

ACT_DTYPE = "bf16"       # referenced by test.py

# blob column offsets (bf16 cols per partition)
W_OFF = 0
CTX_OFF = W_OFF + KT_C * R            # 1024
UB_OFF = CTX_OFF + KT_C * BS          # 3072
VB_OFF = UB_OFF + KT_X * R            # 7168
X_OFF = VB_OFF + RT * UNITS           # 11264
TOT_COLS = X_OFF + KT_X * BS          # 19456

HB = 256                 # stage-2 batch slice width

_COMPILED = {}


def _build(key):
    dt = mybir.dt.bfloat16
    f32 = mybir.dt.float32
    nc = bacc.Bacc("TRN2", target_bir_lowering=False, debug=False,
                   num_devices=N_CORES)

    blob = nc.dram_tensor("blob", [128, TOT_COLS], dt,
                          kind="ExternalInput").ap()
    Bc2 = nc.dram_tensor("Bc2", [128, RT], f32, kind="ExternalInput").ap()
    out = nc.dram_tensor("out", [BS, UNITS], dt, kind="ExternalOutput").ap()

    with tile.TileContext(nc) as tc:
        with (
            tc.tile_pool(name="consts", bufs=1) as consts,
            tc.tile_pool(name="osb", bufs=4) as osb,
            tc.tile_pool(name="ps_h", bufs=RT, space="PSUM") as ps_h,
            tc.tile_pool(name="ps_p", bufs=2, space="PSUM") as ps_p,
            tc.tile_pool(name="ps_o", bufs=4, space="PSUM") as ps_o,
        ):
            Bc_sb = consts.tile([128, RT], f32, tag="bc")
            nc.scalar.dma_start(Bc_sb[:], Bc2[:])
            blob_sb = consts.tile([128, TOT_COLS], dt, tag="blob")
            nc.sync.dma_start(blob_sb[:], blob[:])

            chi_sb = consts.tile([128, RT * BS], f32, tag="chi")
            psT_sb = consts.tile([128, RT * BS], dt, tag="psT")

            # ---- stage 1: h.T = W.T @ ctx.T ; chi = sigmoid(h + Bc) ----
            for rh in range(RT):
                ps = ps_h.tile([128, BS], f32, tag="hps")
                for n in range(KT_C):
                    nc.tensor.matmul(
                        ps[:],
                        blob_sb[:, W_OFF + n * R + rh * 128:
                                W_OFF + n * R + rh * 128 + 128],
                        blob_sb[:, CTX_OFF + n * BS: CTX_OFF + (n + 1) * BS],
                        start=(n == 0), stop=(n == KT_C - 1))
                nc.scalar.activation(
                    chi_sb[:, rh * BS:(rh + 1) * BS], ps[:],
                    mybir.ActivationFunctionType.Sigmoid,
                    bias=Bc_sb[:, rh:rh + 1])

            # ---- stage 2: proj.T per b-slice; psT = proj.T * chi.T ----
            for j in range(BS // HB):
                for rh in range(RT):
                    ps = ps_p.tile([128, HB], f32, tag="pps")
                    for k in range(KT_X):
                        nc.tensor.matmul(
                            ps[:],
                            blob_sb[:, UB_OFF + k * R + rh * 128:
                                    UB_OFF + k * R + rh * 128 + 128],
                            blob_sb[:, X_OFF + k * BS + j * HB:
                                    X_OFF + k * BS + j * HB + HB],
                            start=(k == 0), stop=(k == KT_X - 1))
                    nc.vector.tensor_mul(
                        psT_sb[:, rh * BS + j * HB: rh * BS + j * HB + HB],
                        ps[:],
                        chi_sb[:, rh * BS + j * HB: rh * BS + j * HB + HB])

            # ---- stage 3: out = psT.T @ V.T ; stream stores ----
            for bt in range(NBT):
                o_sb = osb.tile([128, UNITS], dt, tag="o_sb")
                ring = nc.sync if bt % 2 == 0 else nc.scalar
                for q in range(4):
                    ps = ps_o.tile([128, 512], f32, tag="ops")
                    vcol = (q // 2) * 2048 + (q % 2) * 512
                    for rh in range(RT):
                        nc.tensor.matmul(
                            ps[:],
                            psT_sb[:, rh * BS + bt * 128:
                                   rh * BS + bt * 128 + 128],
                            blob_sb[:, VB_OFF + vcol + rh * 1024:
                                    VB_OFF + vcol + rh * 1024 + 512],
                            start=(rh == 0), stop=(rh == RT - 1))
                    dst = o_sb[:, q * 512:(q + 1) * 512]
                    if q % 2:
                        nc.scalar.activation(
                            dst, ps[:], mybir.ActivationFunctionType.Copy)
                    else:
                        nc.vector.tensor_copy(dst, ps[:])
                    if bt == NBT - 1:
                        # finest-grained stores on the last tile: short tail
                        ring.dma_start(
                            out[bt * 128:(bt + 1) * 128,
                                q * 512:(q + 1) * 512], dst)
                    elif q % 2:
                        ring.dma_start(
                            out[bt * 128:(bt + 1) * 128,
                                (q - 1) * 512:(q + 1) * 512],
                            o_sb[:, (q - 1) * 512:(q + 1) * 512])

    nc.compile()
    return nc


def _get_nc(key):
    if key not in _COMPILED:
        _COMPILED[key] = _build(key)
    return _COMPILED[key]


def _pack(a, p=128):
    """[n*p, m] row-major -> [p, n*m]: partition p holds rows p, p+128, ..."""
    n = a.shape[0] // p
    return np.ascontiguousarray(
        a.reshape(n, p, a.shape[1]).transpose(1, 0, 2).reshape(p, -1))


def _prep_in_maps(inputs, context, U, S, V, W, Bc):
    np_act = ml_dtypes.bfloat16

    Us = np.asarray(U, np.float32) * np.asarray(S, np.float32)[None, :]
    ub = _pack(Us).astype(np_act)
    # vb repacked units-half-major: col = uh*2048 + rh*1024 + uu
    vb = _pack(np.ascontiguousarray(np.asarray(V, np.float32).T))
    vb = np.ascontiguousarray(
        vb.reshape(128, RT, 2, UNITS // 2).transpose(0, 2, 1, 3)
          .reshape(128, RT * UNITS)).astype(np_act)
    Wp = _pack(np.asarray(W, np.float32)).astype(np_act)
    Bc2 = np.ascontiguousarray(
        np.asarray(Bc, np.float32).reshape(RT, 128).T)

    x = np.asarray(inputs, np.float32)
    ctx = np.asarray(context, np.float32)
    in_maps = []
    for c in range(N_CORES):
        ctxT = _pack(np.ascontiguousarray(
            ctx[c * BS:(c + 1) * BS, :].T)).astype(np_act)
        xT = _pack(np.ascontiguousarray(
            x[c * BS:(c + 1) * BS, :].T)).astype(np_act)
        blob = np.concatenate([Wp, ctxT, ub, vb, xT], axis=1)
        assert blob.shape == (128, TOT_COLS)
        in_maps.append({"blob": blob, "Bc2": Bc2})
    return in_maps


def kernel(inputs, context, U, S, V, W, Bc, bias, _run_kwargs=None):
    nc = _get_nc("v2")
    in_maps = _prep_in_maps(inputs, context, U, S, V, W, Bc)
    res = run_bass_kernel_spmd(nc, in_maps, list(range(N_CORES)),
                               **(_run_kwargs or {}))
    if _run_kwargs:
        kernel.last_results = res
    out = np.concatenate([np.asarray(res.results[c]["out"]).astype(np.float32)
                          for c in range(N_CORES)], axis=0)
    out += np.asarray(bias, np.float32)[None, :]
    return out


# revision 3
# speedup vs baseline: 1.3467x; 1.3467x over previous
"""Trainium2 Bass kernel for nn_CADenseMul.

Math (see reference):
    chi  = sigmoid(context @ W + Bc)          # [B, R]
    s    = S * chi                            # [B, R]
    out  = ((inputs @ U) * s) @ V.T + bias    # [B, UNITS]

Strategy v3 (makespan-first):
  - Data-parallel over batch B across 8 cores (B=4096 -> 512 rows/core).
  - Host-side prep: transposed shards packed [128, cols]; S folded into U;
    bf16 streams; V.T repacked units-half-major.
  - Loads prioritized & split over both HWDGE rings (sync: W|ctx then x
    b-slices; scalar: Bc, U, V halves) so compute starts ~4us in and
    streams behind the DMA.  Lag-one software pipeline:
        S1 (h/chi) -> S2 j0, S2 j1, S3 b0, S2 j2, S3 b1, S2 j3, S3 b2, S3 b3
  - All output stores on the sync ring; last tile stored in 512-col
    chunks to shorten the tail.
"""

import numpy as np
import ml_dtypes

import concourse.bass as bass
import concourse.tile as tile
from concourse import bacc, mybir
from concourse.bass_utils import run_bass_kernel_spmd

N_CORES = 8
B, D_IN, D_CTX, UNITS, R = 4096, 2048, 512, 2048, 256
BS = B // N_CORES        # 512 batch rows per core
KT_X = D_IN // 128       # 16
KT_C = D_CTX // 128      # 4
RT = R // 128            # 2
NBT = BS // 128          # 4 output batch tiles / stage-2 slices

ACT_DTYPE = "bf16"       # referenced by test.py

_COMPILED = {}


def _build(key):
    dt = mybir.dt.bfloat16
    f32 = mybir.dt.float32
    nc = bacc.Bacc("TRN2", target_bir_lowering=False, debug=False,
                   num_devices=N_CORES)

    wc = nc.dram_tensor("wc", [128, KT_C * R + KT_C * BS], dt,
                        kind="ExternalInput").ap()          # W | ctxT
    ub = nc.dram_tensor("ub", [128, KT_X * R], dt,
                        kind="ExternalInput").ap()          # U_s
    xh = [nc.dram_tensor(f"xh{j}", [128, KT_X * 128], dt,
                         kind="ExternalInput").ap() for j in range(NBT)]
    vb = nc.dram_tensor("vb", [128, RT * UNITS], dt,
                        kind="ExternalInput").ap()          # V.T repacked
    Bc2 = nc.dram_tensor("Bc2", [128, RT], f32, kind="ExternalInput").ap()
    out = nc.dram_tensor("out", [BS, UNITS], dt, kind="ExternalOutput").ap()

    W_off = 0
    ctx_off = KT_C * R

    with tile.TileContext(nc) as tc:
        with (
            tc.tile_pool(name="consts", bufs=1) as consts,
            tc.tile_pool(name="osb", bufs=4) as osb,
            tc.tile_pool(name="ps_h", bufs=RT, space="PSUM") as ps_h,
            tc.tile_pool(name="ps_p", bufs=2, space="PSUM") as ps_p,
            tc.tile_pool(name="ps_o", bufs=4, space="PSUM") as ps_o,
        ):
            # ---- loads: sync ring feeds stage order; scalar ring params ----
            wc_sb = consts.tile([128, KT_C * R + KT_C * BS], dt, tag="wc")
            nc.sync.dma_start(wc_sb[:], wc[:])
            Bc_sb = consts.tile([128, RT], f32, tag="bc")
            nc.scalar.dma_start(Bc_sb[:], Bc2[:])
            ub_sb = consts.tile([128, KT_X * R], dt, tag="ub")
            nc.scalar.dma_start(ub_sb[:], ub[:])
            xh_sb = []
            for j in range(NBT):
                xt = consts.tile([128, KT_X * 128], dt, tag=f"xh{j}")
                xh_sb.append(xt)
            nc.sync.dma_start(xh_sb[0][:], xh[0][:])
            nc.sync.dma_start(xh_sb[1][:], xh[1][:])
            vb_sb = consts.tile([128, RT * UNITS], dt, tag="vb")
            # V halves: units 0:1024 (cols 0:2048) first, then the rest
            nc.scalar.dma_start(vb_sb[:, :RT * 1024], vb[:, :RT * 1024])
            nc.sync.dma_start(xh_sb[2][:], xh[2][:])
            nc.scalar.dma_start(vb_sb[:, RT * 1024:], vb[:, RT * 1024:])
            nc.sync.dma_start(xh_sb[3][:], xh[3][:])

            chi_sb = consts.tile([128, RT * BS], f32, tag="chi")
            psT_sb = consts.tile([128, RT * BS], dt, tag="psT")

            # ---- stage 1: h.T = W.T @ ctx.T ; chi = sigmoid(h + Bc) ----
            for rh in range(RT):
                ps = ps_h.tile([128, BS], f32, tag="hps")
                for n in range(KT_C):
                    nc.tensor.matmul(
                        ps[:],
                        wc_sb[:, W_off + n * R + rh * 128:
                              W_off + n * R + rh * 128 + 128],
                        wc_sb[:, ctx_off + n * BS: ctx_off + (n + 1) * BS],
                        start=(n == 0), stop=(n == KT_C - 1))
                nc.scalar.activation(
                    chi_sb[:, rh * BS:(rh + 1) * BS], ps[:],
                    mybir.ActivationFunctionType.Sigmoid,
                    bias=Bc_sb[:, rh:rh + 1])

            # ---- stage 2 slice j: proj.T ; psT = proj.T * chi.T (bf16) ----
            def emit_s2(j):
                for rh in range(RT):
                    ps = ps_p.tile([128, 128], f32, tag="pps")
                    for k in range(KT_X):
                        nc.tensor.matmul(
                            ps[:],
                            ub_sb[:, k * R + rh * 128: k * R + rh * 128 + 128],
                            xh_sb[j][:, k * 128: (k + 1) * 128],
                            start=(k == 0), stop=(k == KT_X - 1))
                    nc.vector.tensor_mul(
                        psT_sb[:, rh * BS + j * 128: rh * BS + j * 128 + 128],
                        ps[:],
                        chi_sb[:, rh * BS + j * 128: rh * BS + j * 128 + 128])

            # ---- stage 3 tile bt: out rows; stores streamed on sync ----
            def emit_s3(bt):
                o_sb = osb.tile([128, UNITS], dt, tag="o_sb")
                for q in range(4):
                    ps = ps_o.tile([128, 512], f32, tag="ops")
                    vcol = (q // 2) * 2048 + (q % 2) * 512
                    for rh in range(RT):
                        nc.tensor.matmul(
                            ps[:],
                            psT_sb[:, rh * BS + bt * 128:
                                   rh * BS + bt * 128 + 128],
                            vb_sb[:, vcol + rh * 1024:
                                  vcol + rh * 1024 + 512],
                            start=(rh == 0), stop=(rh == RT - 1))
                    dst = o_sb[:, q * 512:(q + 1) * 512]
                    if q % 2:
                        nc.scalar.activation(
                            dst, ps[:], mybir.ActivationFunctionType.Copy)
                    else:
                        nc.vector.tensor_copy(dst, ps[:])
                    if bt == NBT - 1:
                        nc.sync.dma_start(
                            out[bt * 128:(bt + 1) * 128,
                                q * 512:(q + 1) * 512], dst)
                    elif q % 2:
                        nc.sync.dma_start(
                            out[bt * 128:(bt + 1) * 128,
                                (q - 1) * 512:(q + 1) * 512],
                            o_sb[:, (q - 1) * 512:(q + 1) * 512])

            # lag-one pipeline: S3 of slice j runs one slice behind S2
            emit_s2(0)
            emit_s2(1)
            emit_s3(0)
            emit_s2(2)
            emit_s3(1)
            emit_s2(3)
            emit_s3(2)
            emit_s3(3)

    nc.compile()
    return nc


def _get_nc(key):
    if key not in _COMPILED:
        _COMPILED[key] = _build(key)
    return _COMPILED[key]


def _pack(a, p=128):
    """[n*p, m] row-major -> [p, n*m]: partition p holds rows p, p+128, ..."""
    n = a.shape[0] // p
    return np.ascontiguousarray(
        a.reshape(n, p, a.shape[1]).transpose(1, 0, 2).reshape(p, -1))


def _prep_in_maps(inputs, context, U, S, V, W, Bc):
    np_act = ml_dtypes.bfloat16

    Us = np.asarray(U, np.float32) * np.asarray(S, np.float32)[None, :]
    ub = _pack(Us).astype(np_act)
    # vb repacked units-half-major: col = uh*2048 + rh*1024 + uu
    vb = _pack(np.ascontiguousarray(np.asarray(V, np.float32).T))
    vb = np.ascontiguousarray(
        vb.reshape(128, RT, 2, UNITS // 2).transpose(0, 2, 1, 3)
          .reshape(128, RT * UNITS)).astype(np_act)
    W32 = np.asarray(W, np.float32)
    Bc2 = np.ascontiguousarray(
        np.asarray(Bc, np.float32).reshape(RT, 128).T)

    x = np.asarray(inputs, np.float32)
    ctx = np.asarray(context, np.float32)
    in_maps = []
    for c in range(N_CORES):
        ctxT = ctx[c * BS:(c + 1) * BS, :].T
        wcb = np.concatenate([_pack(W32), _pack(np.ascontiguousarray(ctxT))],
                             axis=1).astype(np_act)
        xT = x[c * BS:(c + 1) * BS, :].T
        m = {"wc": wcb, "ub": ub, "vb": vb, "Bc2": Bc2}
        for j in range(NBT):
            m[f"xh{j}"] = _pack(np.ascontiguousarray(
                xT[:, j * 128:(j + 1) * 128])).astype(np_act)
        in_maps.append(m)
    return in_maps


def kernel(inputs, context, U, S, V, W, Bc, bias, _run_kwargs=None):
    nc = _get_nc("v3")
    in_maps = _prep_in_maps(inputs, context, U, S, V, W, Bc)
    res = run_bass_kernel_spmd(nc, in_maps, list(range(N_CORES)),
                               **(_run_kwargs or {}))
    if _run_kwargs:
        kernel.last_results = res
    out = np.concatenate([np.asarray(res.results[c]["out"]).astype(np.float32)
                          for c in range(N_CORES)], axis=0)
    out += np.asarray(bias, np.float32)[None, :]
    return out
